# revision 31
# baseline (speedup 1.0000x reference)
"""Trainium2 Bass kernel for nn_LocallyDense: gather -> 16 group-GEMMs -> Conv1D(k=32) -> LeakyReLU.

Strategy: data-parallel over batch (32 -> 4 per core on 8 cores).
Host staging applies the idx permutation + transpose so the device sees dense
GEMMs only.

  stage 1 (bf16): h[d, (n,b)] = sum_kt W[g]^T x_perm, per group g; the fp32
    PSUM result is split into h_hi (fp8e4m3) + h_lo (fp8e5m2) residual.
  stage 2 (fp8 DoubleRow): conv as GEMM over (tap, d). Each DoubleRow matmul
    contracts both 128-halves of d in one instruction. Three terms recover
    near-bf16 accuracy: h_hi*w_hi + h_lo*w_hi + h_hi*w_lo, with conv weights
    pre-scaled by 64 into the e4m3 range (hi) + e5m2 residual (lo); the 1/64
    is folded out on the host (LeakyReLU is positively homogeneous).
  epilogue: LeakyReLU on DVE, bf16 output, host transposes/unscales.

Group bias b and conv bias are folded into a host-precomputed per-position
bias table added before the activation (they are zero for this problem's
inputs, in which case the add is skipped entirely).
"""
import numpy as np
import ml_dtypes

import concourse.bass as bass
import concourse.mybir as mybir
import concourse.tile as tile
from concourse.alu_op_type import AluOpType
from concourse import bacc
from concourse.bass_utils import run_bass_kernel_spmd

B, N, F, G, S, D = 32, 1024, 512, 16, 64, 256
KC, O = 32, 512            # conv taps, conv out channels
T = N - KC + 1             # 993 valid conv outputs
NCORES = 8
BPC = B // NCORES          # batches per core
FKT = F // 128             # k-tiles over F
NEG_SLOPE = 0.2
WSCALE = 64.0              # conv-weight pre-scale into e4m3 range (power of 2)

F32 = mybir.dt.float32
BF16 = mybir.dt.bfloat16
E4 = mybir.dt.float8e4
E5 = mybir.dt.float8e5
E4np = ml_dtypes.float8_e4m3
E5np = ml_dtypes.float8_e5m2
BFnp = ml_dtypes.bfloat16

# conv output tiles: (t0, nn). First tile reads positions <= 511 only
# (groups 0-7), so it can start while groups 8-15 are still being computed.
JT = [(0, 481), (481, 512)]
# taps that get the h_hi*w_lo correction term (the rest contribute ~1.5e-2
# of the 2e-2 error budget when dropped; each dropped tap saves ~3.3us)
CORR_TAPS = 22

TRACE = False              # test.py flips this to get a profile
STAGES = (1, 2)            # bisect knob: 1 = stage-1 pieces, 2 = conv pieces
PIECES = ("mm", "hi", "lo", "cmm", "act", "dma")   # bisect knob
_cache = {}


def _build(has_bias: bool):
    nc = bacc.Bacc("TRN2", target_bir_lowering=False, debug=False,
                   num_devices=NCORES)
    # x layout: col = (n*BPC + b)*FKT + kt — chunks of positions are flat-
    # contiguous (clean subtile deps), stage-1 rhs is a stride-FKT AP.
    xp_d = nc.dram_tensor("xp", [128, N * BPC * FKT], BF16,
                          kind="ExternalInput").ap()
    w_d = nc.dram_tensor("w", [4, 128, 4 * FKT * D], BF16,
                         kind="ExternalInput").ap()
    cwh_d = nc.dram_tensor("cwh", [4, 128, KC * 2 * 128], E4,
                           kind="ExternalInput").ap()
    cwl_d = nc.dram_tensor("cwl", [4, 128, KC * 2 * 128], E5,
                           kind="ExternalInput").ap()
    y_d = nc.dram_tensor("y", [BPC, O, T], BF16, kind="ExternalOutput").ap()
    if has_bias:
        bc_d = nc.dram_tensor("bc", [4, 128, T], F32, kind="ExternalInput").ap()

    with tile.TileContext(nc) as tc:
        with tc.tile_pool(name="x", bufs=4) as p_x, \
             tc.tile_pool(name="wg", bufs=4) as p_w, \
             tc.tile_pool(name="ht", bufs=1) as p_ht, \
             tc.tile_pool(name="cw", bufs=1) as p_cw, \
             tc.tile_pool(name="bias", bufs=1) as p_bias, \
             tc.tile_pool(name="yout", bufs=8) as p_out, \
             tc.tile_pool(name="ps1", bufs=4, space="PSUM") as p_ps1, \
             tc.tile_pool(name="ps2", bufs=4, space="PSUM") as p_ps2:

            # ---- tiles ----
            x_sb = p_x.tile([128, FKT * N * BPC], BF16, tag="x", name="x_sb",
                            bufs=1)
            x_v = x_sb[:].rearrange("p (c kt) -> p c kt", kt=FKT)
            w_sb = [p_w.tile([128, 4 * FKT * D], BF16, tag="w", name=f"w{wt}")
                    for wt in range(4)]
            # h layout: col = dh*(N*BPC) + n*BPC + b  (group ranges contiguous)
            ht_hi = p_ht.tile([128, 2 * N * BPC], E4, tag="hh", name="ht_hi")
            ht_lo = p_ht.tile([128, 2 * N * BPC], E5, tag="hl", name="ht_lo")
            cwh_sb = [p_cw.tile([128, KC * 2 * 128], E4, tag=f"cwh{m}",
                                name=f"cwh{m}") for m in range(4)]
            cwl_sb = [p_cw.tile([128, KC * 2 * 128], E5, tag=f"cwl{m}",
                                name=f"cwl{m}") for m in range(4)]
            if has_bias:
                bc_sb = [p_bias.tile([128, T], F32, tag=f"bc{m}", name=f"bc{m}")
                         for m in range(4)]

            hh_w = ht_hi[:].rearrange("p (dh n b) -> p dh n b", dh=2, b=BPC)
            hl_w = ht_lo[:].rearrange("p (dh n b) -> p dh n b", dh=2, b=BPC)
            cwh_v = [t[:].rearrange("p (tap dh o) -> p tap dh o", tap=KC, dh=2)
                     for t in cwh_sb]
            cwl_v = [t[:].rearrange("p (tap dh o) -> p tap dh o", tap=KC, dh=2)
                     for t in cwl_sb]

            HALF = N * BPC // 2

            def stage1(g):
                if 1 not in STAGES:
                    return
                for m in range(2):
                    ps = p_ps1.tile([128, BPC * S], F32, tag="ps1", name=f"ps1_{g}_{m}")
                    for kt in range(FKT):
                        base = (g % 4) * FKT * D + kt * D + m * 128
                        lhsT = w_sb[g // 4][:, base: base + 128]
                        rhs = x_v[:, g * S * BPC:(g + 1) * S * BPC, kt]
                        nc.tensor.matmul(ps[:], lhsT, rhs,
                                         start=(kt == 0), stop=(kt == FKT - 1))
                    psv = ps[:].rearrange("p (n b) -> p n b", b=BPC)
                    hi = hh_w[:, m, g * S:(g + 1) * S, :]
                    lo = hl_w[:, m, g * S:(g + 1) * S, :]
                    if "hi" in PIECES:
                        nc.scalar.copy(hi, psv)                   # rounds to e4m3
                    if "lo" in PIECES:
                        nc.vector.scalar_tensor_tensor(           # lo = ps - hi
                            lo, psv, 1.0, hi, AluOpType.mult, AluOpType.subtract)

            def conv_tile(mo, bb, t0, nn):
                if 2 not in STAGES:
                    return
                ps = p_ps2.tile([128, 512], F32, tag="ps2", name=f"ps2_{mo}_{bb}_{t0}")
                ops = []
                for hv, wv, ntap in ((hh_w, cwh_v, KC), (hl_w, cwh_v, KC),
                                     (hh_w, cwl_v, CORR_TAPS)):
                    for tap in range(ntap):
                        # rhs [p, dh(2), nn]: dh stride N*BPC, n stride BPC
                        rhs = hv[:, :, t0 + tap: t0 + tap + nn, bb]
                        ops.append((wv[mo][:, tap], rhs))
                if "cmm" in PIECES:
                    for i, (l, r) in enumerate(ops):
                        nc.tensor.matmul(ps[:, :nn], l, r, start=(i == 0),
                                         stop=(i == len(ops) - 1),
                                         perf_mode=mybir.MatmulPerfMode.DoubleRow)
                if has_bias:
                    nc.vector.tensor_tensor(ps[:, :nn], ps[:, :nn],
                                            bc_sb[mo][:, t0:t0 + nn],
                                            AluOpType.add)
                y_sb = p_out.tile([128, 512], BF16, tag="y", name=f"y_{mo}_{bb}_{t0}")
                if "act" in PIECES:
                    nc.scalar.copy(y_sb[:, :nn], ps[:, :nn])  # ACT: psum -> bf16
                    nc.vector.scalar_tensor_tensor(           # LeakyReLU in-place
                        y_sb[:, :nn], y_sb[:, :nn], NEG_SLOPE, y_sb[:, :nn],
                        AluOpType.mult, AluOpType.max)
                if "dma" in PIECES:
                    nc.sync.dma_start(y_d[bb, mo * 128:(mo + 1) * 128, t0:t0 + nn],
                                      y_sb[:, :nn])

            # ---- DMA + compute emission (order == DMA priority) ----
            def xchunk(c0, c1):                       # x cols [c0,c1) of all kt
                nc.sync.dma_start(x_sb[:, c0 * FKT:c1 * FKT],
                                  xp_d[:, c0 * FKT:c1 * FKT])

            xchunk(0, 512)                            # groups 0-1
            nc.sync.dma_start(w_sb[0][:], w_d[0])     # w groups 0-3
            xchunk(512, 1024)                         # groups 2-3
            xchunk(1024, 2048)                        # groups 4-7
            nc.sync.dma_start(w_sb[1][:], w_d[1])     # w groups 4-7
            nc.sync.dma_start(cwh_sb[0][:], cwh_d[0])
            nc.sync.dma_start(cwl_sb[0][:], cwl_d[0])
            xchunk(2048, 3072)                        # groups 8-11
            nc.sync.dma_start(w_sb[2][:], w_d[2])     # w groups 8-11
            xchunk(3072, 4096)                        # groups 12-15
            nc.sync.dma_start(w_sb[3][:], w_d[3])     # w groups 12-15
            for mo in range(1, 4):
                nc.sync.dma_start(cwh_sb[mo][:], cwh_d[mo])
                nc.sync.dma_start(cwl_sb[mo][:], cwl_d[mo])
            if has_bias:
                for m in range(4):
                    nc.sync.dma_start(bc_sb[m][:], bc_d[m])

            # PE p-state warm-up: harmless tiny matmuls on a zeroed tile keep
            # the tensor engine busy (and its clock ramped) while the first
            # input DMAs land and across the stage-1 -> conv handoffs.
            warm_sb = p_x.tile([128, 64], BF16, tag="warm", name="warm_sb",
                               bufs=1)
            nc.vector.memset(warm_sb[:], 0)

            def pewarm(n, label):
                ps = p_ps1.tile([128, BPC * S], F32, tag="ps1",
                                name=f"warm_{label}")
                for i in range(n):
                    nc.tensor.matmul(ps[:64, :64], warm_sb[:, :64],
                                     warm_sb[:, :64], start=True, stop=True)

            pewarm(100, "boot")

            for g in range(8):                        # stage 1, groups 0-7
                stage1(g)

            pewarm(24, "bridge_a")                    # bridge the h-chain wait

            t0, nn = JT[0]                            # conv-A (needs groups 0-7)
            for mo in range(4):
                for bb in range(BPC):
                    conv_tile(mo, bb, t0, nn)

            for g in range(8, G):                     # stage 1, groups 8-15
                stage1(g)

            pewarm(24, "bridge_b")                    # bridge the h-chain wait

            t0, nn = JT[1]                            # conv-B (needs all groups)
            for mo in range(4):
                for bb in range(BPC):
                    if mo == 3 and bb == BPC - 1:
                        # split the final tile so the kernel tail (epilogue +
                        # store after the last matmul) is short
                        conv_tile(mo, bb, t0, 384)
                        conv_tile(mo, bb, t0 + 384, nn - 384)
                    else:
                        conv_tile(mo, bb, t0, nn)

    nc.compile()
    return nc


def kernel(x, idx, W, b, conv_w, conv_b):
    x = np.asarray(x); idx = np.asarray(idx); W = np.asarray(W)
    b = np.asarray(b); conv_w = np.asarray(conv_w); conv_b = np.asarray(conv_b)
    has_bias = bool(np.any(b) or np.any(conv_b))
    key = ("nc", has_bias)
    if key not in _cache:
        _cache[key] = _build(has_bias)
        _cache["nc"] = _cache[key]   # for test.py's TimelineSim hook
    nc = _cache[key]

    idx_flat = idx.reshape(-1).astype(np.int64)
    # x permuted + transposed: xp[c][p, n, b, kt] = x[4c+b, idx_flat[n], 128kt+p]
    xg = x[:, idx_flat, :].astype(BFnp)                       # [B, N, F]
    xp = np.ascontiguousarray(
        xg.transpose(2, 1, 0).reshape(FKT, 128, N, NCORES, BPC)
        .transpose(3, 1, 2, 4, 0)).reshape(NCORES, 128, N * BPC * FKT)

    # stage-1 weights, 4 groups per tile: wq[wt, p, gi, kt, d] = W[4wt+gi, 128kt+p, d]
    wq = np.ascontiguousarray(
        W.astype(BFnp).reshape(4, 4, FKT, 128, D).transpose(0, 3, 1, 2, 4)
    ).reshape(4, 128, 4 * FKT * D)

    # conv weights scaled into e4m3 range + e5m2 residual:
    # cw[mo, p, tap, dh, o] = conv_w[tap, 128dh+p, 128mo+o] * 64
    cws = (conv_w * np.float32(WSCALE)).reshape(KC, 2, 128, 4, 128)
    cws = np.ascontiguousarray(cws.transpose(3, 2, 0, 1, 4))  # [mo,p,tap,dh,o]
    cwh = cws.astype(E4np)
    cwl = (cws - cwh.astype(np.float32)).astype(E5np)
    cwh = cwh.reshape(4, 128, KC * 2 * 128)
    cwl = cwl.reshape(4, 128, KC * 2 * 128)

    in_maps = []
    for c in range(NCORES):
        m = {"xp": xp[c], "w": wq, "cwh": cwh, "cwl": cwl}
        if has_bias:
            # bias_conv[t, o] = sum_tap b[g(t+tap)] @ conv_w[tap] + conv_b
            M = np.einsum('gd,tdo->gto', b, conv_w, optimize=True)  # [G, KC, O]
            P = np.concatenate([np.zeros((G, 1, O), np.float32),
                                np.cumsum(M, axis=1)], axis=1)      # [G, KC+1, O]
            t = np.arange(T)
            q, r = t >> 6, t & 63
            j1 = np.minimum(64 - r, KC)
            bc = P[q, j1] + (P[np.minimum(q + 1, G - 1), KC]
                             - P[np.minimum(q + 1, G - 1), j1]) * (j1 < KC)[:, None]
            bc = (bc + conv_b[None, :]) * np.float32(WSCALE)        # [T, O]
            m["bc"] = np.ascontiguousarray(
                bc.T.reshape(4, 128, T)).astype(np.float32)
        in_maps.append(m)

    res = run_bass_kernel_spmd(nc, in_maps, core_ids=list(range(NCORES)),
                               trace=TRACE)
    if TRACE and res.exec_time_ns is not None:
        print(f"HW exec time: {res.exec_time_ns} ns")
        if res.instructions_and_trace is not None:
            print("trace:", res.instructions_and_trace[1])
    y = np.stack([r["y"] for r in res.results])       # [NC, BPC, O, T] bf16
    y = y.reshape(B, O, T).transpose(0, 2, 1).astype(np.float32)
    return np.ascontiguousarray(y * np.float32(1.0 / WSCALE))
